# revision 8
# baseline (speedup 1.0000x reference)
"""NT-Xent contrastive loss (SimCLR) on 8 Trainium2 NeuronCores — v4.

z1, z2 [4096, 256] fp32 -> scalar loss.
  zn = l2norm(concat(z1, z2))            [8192, 256]
  sim = zn @ zn.T / 0.07                 [8192, 8192]
  loss = -mean_i log_softmax(sim)[i, (i + 4096) % 8192]

Exploits symmetry of exp(sim): each unordered 128x128 block pair computed
ONCE. Row sums via ACT exp accumulator / DVE reduce (free axis); transpose
sums via PE matmuls (exp tile stationary x ones vector, partition axis).
Work split across engines: ACT does 2 strips per q-row (exact exp), DVE does
the tail strip with an int16/bf16 exp2 bit-trick.

Distribution (circulant, SPMD-uniform): 64 key blocks; global row q covers
ring range [q, q+L) mod 64, L=33 for q<32 else 32 — every unordered pair
exactly once. Core c owns global rows {c, c+8, ..., c+56}; after rolling the
key space by 128*c cols every core runs the IDENTICAL program on local
q-rows {0,8,...,56}. Keys are ring-unrolled (padded to 88 blocks) and
supplied as 4 contiguous chunk tensors for fast DMA.

Gram matmuls: fp8 e4m3 (z scaled by 16), DoubleRow (K=256 in one pass).
Outputs per core: rowout [128,24] strip row-sums, colout [128,88] col-sum
partials, tgtout [128,4] target-pair sims. Host reduces, logs, averages.
"""

import math

import numpy as np

import concourse.bass as bass
import concourse.tile as tile
from concourse import bacc, mybir
from concourse.bass_utils import run_bass_kernel_spmd

B, D = 4096, 256
N = 2 * B
NCORES = 8
NB = N // 128                 # 64 key blocks
TEMP = 0.07
ZSCALE = 16.0                 # z pre-scaled by 16 -> s' = 256 * s_true

QROWS = [0, 8, 16, 24, 32, 40, 48, 56]   # local q-row blocks (all cores)
LS = [33, 33, 33, 33, 32, 32, 32, 32]    # ring length (blocks) per slot
PADB = 88                                 # local key blocks incl ring pad
PADW = PADB * 128                         # 11264 padded key cols
CHUNKS = [(0, 1536), (1536, 1536), (3072, 3072), (6144, 3072), (9216, 2048)]

F32 = mybir.dt.float32
BF16 = mybir.dt.bfloat16
I16 = mybir.dt.int16
F8 = mybir.dt.float8e4
NP_F8 = mybir.dt.np(F8)
NP_BF16 = mybir.dt.np(BF16)

STRIPW = 1536
# per-q-row strips: [0]=ACT, [1]=ACT, [2]=DVE bit-trick (tail-only)
DVE_STRIP = 2

# exp2 bit-trick: round(MUL*s' + ADD) as int16, bitcast bf16 ~=
# exp((s'/256-1)/T); rel err within +-4%, mean ~1e-4 — tail strips only.
_LOG2E = math.log2(math.e)
TRICK_MUL = 128.0 * _LOG2E / (256.0 * TEMP)
TRICK_ADD = 128.0 * (126.94269504 - _LOG2E / TEMP)


def strips_of(L):
    w, out = L * 128, []
    while w > 0:
        out.append(min(STRIPW, w))
        w -= out[-1]
    return out


N_SLOTS = 4 * len(QROWS)  # 4 row-sum slots per q-row


def contrib_tables():
    first, last = {}, {}
    for ql, L in zip(QROWS, LS):
        for kb in range(ql + 1, ql + L):
            if kb not in first:
                first[kb] = ql
            last[kb] = ql
    assert sorted(first) == list(range(1, PADB))
    return first, last


FIRST_Q, LAST_Q = contrib_tables()


def build_nc() -> bass.Bass:
    nc = bacc.Bacc("TRN2", target_bir_lowering=False, debug=False, num_devices=NCORES)
    zparams = [
        nc.declare_dram_parameter(f"zt8c{g}", [128, 2, cw], F8, isOutput=False)
        for g, (c0, cw) in enumerate(CHUNKS)
    ]
    cstf = nc.declare_dram_parameter("cstf", [128, 1], F32, isOutput=False)
    identb = nc.declare_dram_parameter("identb", [128, 128], BF16, isOutput=False)
    ones = nc.declare_dram_parameter("ones", [128, 1], BF16, isOutput=False)
    rowout = nc.declare_dram_parameter("rowout", [128, N_SLOTS], F32, isOutput=True)
    tgtout = nc.declare_dram_parameter("tgtout", [128, 4], F32, isOutput=True)
    colout = nc.declare_dram_parameter("colout", [128, PADB], F32, isOutput=True)

    act_scale = 1.0 / (ZSCALE * ZSCALE * TEMP)

    with tile.TileContext(nc) as tc:
        with (
            tc.tile_pool(name="zt", bufs=1) as zt_pool,
            tc.tile_pool(name="const", bufs=1) as const_pool,
            tc.tile_pool(name="stats", bufs=1) as stats_pool,
            tc.tile_pool(name="escr", bufs=3) as e_pool,
            tc.tile_pool(name="e16", bufs=3) as e16_pool,
            tc.tile_pool(name="dscr", bufs=2) as d_pool,
            tc.tile_pool(name="gram", bufs=2, space="PSUM") as gram_pool,
            tc.tile_pool(name="cacc", bufs=1, space="PSUM") as cacc_pool,
            tc.tile_pool(name="warmp", bufs=1, space="PSUM") as warm_pool,
        ):
            # chunk 0 first: compute can't start without it
            zchunk = [None] * len(CHUNKS)
            zchunk[0] = zt_pool.tile([128, 2, CHUNKS[0][1]], F8, tag="zt0", name="zt0")
            nc.sync.dma_start(zchunk[0][:], zparams[0][:])

            bias_t = const_pool.tile([128, 1], F32)
            nc.sync.dma_start(bias_t[:], cstf[:])
            identb_t = const_pool.tile([128, 128], BF16)
            nc.sync.dma_start(identb_t[:], identb[:])
            ones_t = const_pool.tile([128, 1], BF16)
            nc.sync.dma_start(ones_t[:], ones[:])

            for gi in range(1, len(CHUNKS)):
                t = zt_pool.tile(
                    [128, 2, CHUNKS[gi][1]], F8, tag=f"zt{gi}", name=f"zt{gi}"
                )
                nc.sync.dma_start(t[:], zparams[gi][:])
                zchunk[gi] = t

            tgt_t = stats_pool.tile([128, 4], F32)
            colacc = cacc_pool.tile([128, PADB], F32)

            def chunk_of(gstart):
                for ci, (c0, cw) in enumerate(CHUNKS):
                    if c0 <= gstart < c0 + cw:
                        return ci
                raise AssertionError(gstart)

            def chunk_ap(gstart, width):
                ci = chunk_of(gstart)
                off = gstart - CHUNKS[ci][0]
                assert off + width <= CHUNKS[ci][1], (gstart, width)
                return zchunk[ci][:, :, off:off + width]

            def emit_gram(ps, base, off, w_st, stat_ap):
                loc = 0
                while loc < w_st:
                    seg = min(512, w_st - loc)
                    nc.tensor.matmul(
                        ps[:, loc:loc + seg],
                        lhsT=stat_ap,
                        rhs=chunk_ap(base + off + loc, seg),
                        start=True,
                        stop=True,
                        perf_mode=mybir.MatmulPerfMode.DoubleRow,
                    )
                    loc += seg

            def emit_colsum(echunk, ql, off, w_st):
                for j in range(w_st // 128):
                    kb = ql + (off + j * 128) // 128
                    if kb == ql:
                        continue  # diagonal block: row-sums already cover it
                    nc.tensor.matmul(
                        colacc[:, kb:kb + 1],
                        lhsT=echunk(j),
                        rhs=ones_t[:, 0:1],
                        start=(FIRST_Q[kb] == ql),
                        stop=(LAST_Q[kb] == ql),
                    )

            # PE warm-up + filler: junk matmuls into a dead psum bank keep the
            # PE HAM clock gate at 2.4 GHz (it throttles to 1.2 GHz whenever
            # the activity monitor sees an idle-ish 3.4us window, doubling
            # every gram matmul). Source is a memset tile (no DMA dep).
            wsrc = const_pool.tile([128, 512], BF16)
            nc.gpsimd.memset(wsrc[:], 0)
            warm = warm_pool.tile([128, 512], F32)

            def junk(n):
                for _ in range(n):
                    nc.tensor.matmul(
                        warm[:, :512],
                        lhsT=wsrc[:, 0:128],
                        rhs=wsrc[:],
                        start=True,
                        stop=True,
                    )

            junk(12)

            # Per q-row: 4 psum tenants — 2 ACT strips (exact exp) + 2 DVE
            # sub-strips (exp2 bit-trick). With a 2-slot psum pool the
            # boundary tile's slot is freed by an EARLY DVE tensor_scalar,
            # not the late exp, so ACT runs back-to-back across rows.
            # Late-dependency colsums are carried into the next row's PE
            # stream (colacc first/last flags are per (kb, q-row), so the
            # cross-row emission order per column is preserved).
            carried = []
            for s, (ql, L) in enumerate(zip(QROWS, LS)):
                base = ql * 128
                stat_ap = chunk_ap(base, 128)
                rowq = stats_pool.tile([128, 4], F32, tag=f"rowq{s}")
                w0 = w1 = STRIPW
                rem = L * 128 - 2 * STRIPW          # 1152 or 1024
                # both sub-strip offsets stay 512-aligned w.r.t. the chunk grid
                wD1, wD2 = (1024, 128) if rem == 1152 else (512, 512)
                o0, o1, oD1, oD2 = 0, w0, 2 * STRIPW, 2 * STRIPW + wD1

                # --- ACT strip 0 (boundary: slot freed by prev row's TS1) ---
                ps0 = gram_pool.tile([128, STRIPW], F32, tag="ps", name="ps0")
                emit_gram(ps0, base, o0, w0, stat_ap)
                et0 = e_pool.tile([128, STRIPW], BF16, tag="E", name="et0")
                nc.scalar.activation(
                    et0[:], ps0[:], mybir.ActivationFunctionType.Exp,
                    scale=act_scale, bias=bias_t[:],
                    accum_out=rowq[:, 0:1],
                )

                # --- ACT strip 1 ---
                ps1 = gram_pool.tile([128, STRIPW], F32, tag="ps", name="ps1")
                emit_gram(ps1, base, o1, w1, stat_ap)
                for work in carried:
                    work()
                carried = []
                junk(0)
                et1 = e_pool.tile([128, STRIPW], BF16, tag="E", name="et1")
                nc.scalar.activation(
                    et1[:], ps1[:], mybir.ActivationFunctionType.Exp,
                    scale=act_scale, bias=bias_t[:],
                    accum_out=rowq[:, 1:2],
                )

                # --- DVE sub-strip 2 FIRST (short TS frees its slot fast;
                # that slot is the next row's boundary gram slot) ---
                psD2 = gram_pool.tile([128, STRIPW], F32, tag="ps", name="psD2")
                emit_gram(psD2, base, oD2, wD2, stat_ap)
                e16b = e16_pool.tile([128, 512], I16, tag="E16b")
                nc.vector.tensor_scalar(
                    e16b[:, :wD2], psD2[:, :wD2],
                    TRICK_MUL, TRICK_ADD,
                    mybir.AluOpType.mult, mybir.AluOpType.add,
                )
                nc.vector.tensor_reduce(
                    out=rowq[:, 3:4], in_=e16b[:, :wD2].bitcast(BF16),
                    axis=mybir.AxisListType.X, op=mybir.AluOpType.add,
                )
                emit_colsum(lambda j: et0[:, j * 128:(j + 1) * 128], ql, o0, w0)

                # --- DVE sub-strip 1 (long; spills into the next row) ---
                psD1 = gram_pool.tile([128, STRIPW], F32, tag="ps", name="psD1")
                emit_gram(psD1, base, oD1, wD1, stat_ap)
                e16a = e16_pool.tile([128, 1024], I16, tag="E16a")
                nc.vector.tensor_scalar(
                    e16a[:, :wD1], psD1[:, :wD1],
                    TRICK_MUL, TRICK_ADD,
                    mybir.AluOpType.mult, mybir.AluOpType.add,
                )
                nc.vector.tensor_reduce(
                    out=rowq[:, 2:3], in_=e16a[:, :wD1].bitcast(BF16),
                    axis=mybir.AxisListType.X, op=mybir.AluOpType.add,
                )
                # target pairs: diagonal of tile (q, q+32) = DVE sub-strip 2;
                # s' recovered on host from the exp bits
                if s < 4:
                    dg = d_pool.tile([128, 128], BF16, tag="dg")
                    nc.vector.tensor_mul(
                        dg[:], e16b[:, 0:128].bitcast(BF16), identb_t[:]
                    )
                    nc.vector.tensor_reduce(
                        out=tgt_t[:, s:s + 1], in_=dg[:],
                        axis=mybir.AxisListType.X, op=mybir.AluOpType.add,
                    )

                def make_carry(et1=et1, e16a=e16a, e16b=e16b, ql=ql, o1=o1,
                               w1=w1, oD1=oD1, wD1=wD1, oD2=oD2, wD2=wD2):
                    return [
                        lambda: emit_colsum(
                            lambda j: et1[:, j * 128:(j + 1) * 128], ql, o1, w1
                        ),
                        lambda: emit_colsum(
                            lambda j: e16a[:, j * 128:(j + 1) * 128].bitcast(BF16),
                            ql, oD1, wD1,
                        ),
                        lambda: emit_colsum(
                            lambda j: e16b[:, j * 128:(j + 1) * 128].bitcast(BF16),
                            ql, oD2, wD2,
                        ),
                    ]

                carried = make_carry()

                nc.sync.dma_start(rowout[:, s * 4:(s + 1) * 4], rowq[:])
                if s == 3:
                    nc.sync.dma_start(tgtout[:], tgt_t[:])

            for work in carried:
                work()

            colsb = stats_pool.tile([128, PADB], F32)
            nc.vector.tensor_copy(colsb[:], colacc[:])
            nc.sync.dma_start(colout[:], colsb[:])

    nc.compile()
    return nc


_NC_CACHE: list = []


def _get_nc():
    if not _NC_CACHE:
        _NC_CACHE.append(build_nc())
    return _NC_CACHE[0]


def _prepare_inputs(z1: np.ndarray, z2: np.ndarray) -> list[dict]:
    z = np.concatenate([np.asarray(z1), np.asarray(z2)], axis=0).astype(np.float32)
    zn = z / np.maximum(np.linalg.norm(z, axis=1, keepdims=True), 1e-12)
    # DoubleRow layout: [128 partitions, 2 k-slabs, 8192 keys], scaled by 16
    zt = np.ascontiguousarray((zn.T * ZSCALE).reshape(2, 128, N).transpose(1, 0, 2))
    zt8 = zt.astype(NP_F8)
    ones = np.ones((128, 1), dtype=NP_BF16)
    biasv = np.full((128, 1), -1.0 / TEMP, dtype=np.float32)
    identb = np.eye(128, dtype=NP_BF16)
    in_maps = []
    for c in range(NCORES):
        rolled = np.roll(zt8, -128 * c, axis=2)
        padded = np.concatenate([rolled, rolled[:, :, : PADW - N]], axis=2)
        m = {"cstf": biasv, "identb": identb, "ones": ones}
        for g, (c0, cw) in enumerate(CHUNKS):
            m[f"zt8c{g}"] = np.ascontiguousarray(padded[:, :, c0:c0 + cw])
        in_maps.append(m)
    return in_maps


def _reduce_outputs(results) -> np.float32:
    R = np.zeros(N, dtype=np.float64)
    tgt = np.empty(N, dtype=np.float64)
    for c in range(NCORES):
        rowacc = results[c]["rowout"].astype(np.float64)   # [128, N_SLOTS]
        colacc = results[c]["colout"].astype(np.float64)   # [128, PADB]
        tgt_c = results[c]["tgtout"].astype(np.float64)    # [128, 4]
        for s, ql in enumerate(QROWS):
            g = ql + c   # global q block (<= 63)
            r = rowacc[:, s * 4:(s + 1) * 4].sum(axis=1)
            R[g * 128:(g + 1) * 128] += r
        for kb in range(1, PADB):
            gk = (kb + c) % NB
            R[gk * 128:(gk + 1) * 128] += colacc[:, kb]
        for s in range(4):
            gq = QROWS[s] + c   # < 32
            rows = gq * 128 + np.arange(128)
            # tgt_c holds exp((s'/256-1)/T) bits summed over the diagonal
            sp = 256.0 + 256.0 * TEMP * np.log(np.maximum(tgt_c[:, s], 1e-300))
            tgt[rows] = sp
            tgt[rows + B] = sp
    logS = 1.0 / TEMP + np.log(R)
    loss = np.mean(logS - tgt / (ZSCALE * ZSCALE * TEMP))
    return np.float32(loss)


def _run(z1, z2, trace=False):
    nc = _get_nc()
    in_maps = _prepare_inputs(z1, z2)
    res = run_bass_kernel_spmd(nc, in_maps, list(range(NCORES)), trace=trace)
    return _reduce_outputs(res.results), res


def kernel(z1: np.ndarray, z2: np.ndarray) -> np.ndarray:
    loss, _ = _run(z1, z2, trace=False)
    return loss


if __name__ == "__main__":
    rng = np.random.default_rng(0)
    z1 = rng.standard_normal((B, D), dtype=np.float32)
    z2 = rng.standard_normal((B, D), dtype=np.float32)
    print(kernel(z1, z2))


# revision 11
# speedup vs baseline: 1.0423x; 1.0423x over previous
"""NT-Xent contrastive loss (SimCLR) on 8 Trainium2 NeuronCores — v4.

z1, z2 [4096, 256] fp32 -> scalar loss.
  zn = l2norm(concat(z1, z2))            [8192, 256]
  sim = zn @ zn.T / 0.07                 [8192, 8192]
  loss = -mean_i log_softmax(sim)[i, (i + 4096) % 8192]

Exploits symmetry of exp(sim): each unordered 128x128 block pair computed
ONCE. Row sums via ACT exp accumulator / DVE reduce (free axis); transpose
sums via PE matmuls (exp tile stationary x ones vector, partition axis).
Work split across engines: ACT does 2 strips per q-row (exact exp), DVE does
the tail strip with an int16/bf16 exp2 bit-trick.

Distribution (circulant, SPMD-uniform): 64 key blocks; global row q covers
ring range [q, q+L) mod 64, L=33 for q<32 else 32 — every unordered pair
exactly once. Core c owns global rows {c, c+8, ..., c+56}; after rolling the
key space by 128*c cols every core runs the IDENTICAL program on local
q-rows {0,8,...,56}. Keys are ring-unrolled (padded to 88 blocks) and
supplied as 4 contiguous chunk tensors for fast DMA.

Gram matmuls: fp8 e4m3 (z scaled by 16), DoubleRow (K=256 in one pass).
Outputs per core: rowout [128,24] strip row-sums, colout [128,88] col-sum
partials, tgtout [128,4] target-pair sims. Host reduces, logs, averages.
"""

import math

import numpy as np

import concourse.bass as bass
import concourse.tile as tile
from concourse import bacc, mybir
from concourse.bass_utils import run_bass_kernel_spmd

B, D = 4096, 256
N = 2 * B
NCORES = 8
NB = N // 128                 # 64 key blocks
TEMP = 0.07
ZSCALE = 16.0                 # z pre-scaled by 16 -> s' = 256 * s_true

QROWS = [0, 8, 16, 24, 32, 40, 48, 56]   # local q-row blocks (all cores)
LS = [33, 33, 33, 33, 32, 32, 32, 32]    # ring length (blocks) per slot
PADB = 88                                 # local key blocks incl ring pad
PADW = PADB * 128                         # 11264 padded key cols
CHUNKS = [(0, 1536), (1536, 1536), (3072, 3072), (6144, 3072), (9216, 2048)]

F32 = mybir.dt.float32
BF16 = mybir.dt.bfloat16
I16 = mybir.dt.int16
F8 = mybir.dt.float8e4
NP_F8 = mybir.dt.np(F8)
NP_BF16 = mybir.dt.np(BF16)

STRIPW = 1536
# per-q-row strips: [0]=ACT, [1]=ACT, [2]=DVE bit-trick (tail-only)
DVE_STRIP = 2

# exp2 bit-trick: round(MUL*s' + ADD) as int16, bitcast bf16 ~=
# exp((s'/256-1)/T); rel err within +-4%, mean ~1e-4 — tail strips only.
_LOG2E = math.log2(math.e)
TRICK_MUL = 128.0 * _LOG2E / (256.0 * TEMP)
TRICK_ADD = 128.0 * (126.94269504 - _LOG2E / TEMP)


def strips_of(L):
    w, out = L * 128, []
    while w > 0:
        out.append(min(STRIPW, w))
        w -= out[-1]
    return out


N_SLOTS = 4 * len(QROWS)  # 4 row-sum slots per q-row


def contrib_tables():
    first, last = {}, {}
    for ql, L in zip(QROWS, LS):
        for kb in range(ql + 1, ql + L):
            if kb not in first:
                first[kb] = ql
            last[kb] = ql
    assert sorted(first) == list(range(1, PADB))
    return first, last


FIRST_Q, LAST_Q = contrib_tables()


def build_nc() -> bass.Bass:
    nc = bacc.Bacc("TRN2", target_bir_lowering=False, debug=False, num_devices=NCORES)
    zparams = [
        nc.declare_dram_parameter(f"zt8c{g}", [128, 2, cw], F8, isOutput=False)
        for g, (c0, cw) in enumerate(CHUNKS)
    ]
    cstf = nc.declare_dram_parameter("cstf", [128, 1], F32, isOutput=False)
    identb = nc.declare_dram_parameter("identb", [128, 128], BF16, isOutput=False)
    ones = nc.declare_dram_parameter("ones", [128, 1], BF16, isOutput=False)
    rowout = nc.declare_dram_parameter("rowout", [128, N_SLOTS], F32, isOutput=True)
    tgtout = nc.declare_dram_parameter("tgtout", [128, 4], F32, isOutput=True)
    colout = nc.declare_dram_parameter("colout", [128, PADB], F32, isOutput=True)

    act_scale = 1.0 / (ZSCALE * ZSCALE * TEMP)

    with tile.TileContext(nc) as tc:
        with (
            tc.tile_pool(name="zt", bufs=1) as zt_pool,
            tc.tile_pool(name="const", bufs=1) as const_pool,
            tc.tile_pool(name="stats", bufs=1) as stats_pool,
            tc.tile_pool(name="escr", bufs=3) as e_pool,
            tc.tile_pool(name="e16", bufs=3) as e16_pool,
            tc.tile_pool(name="dscr", bufs=2) as d_pool,
            tc.tile_pool(name="gram", bufs=2, space="PSUM") as gram_pool,
            tc.tile_pool(name="cacc", bufs=1, space="PSUM") as cacc_pool,
            tc.tile_pool(name="warmp", bufs=1, space="PSUM") as warm_pool,
        ):
            # chunk 0 first: compute can't start without it
            zchunk = [None] * len(CHUNKS)
            zchunk[0] = zt_pool.tile([128, 2, CHUNKS[0][1]], F8, tag="zt0", name="zt0")
            nc.sync.dma_start(zchunk[0][:], zparams[0][:])

            bias_t = const_pool.tile([128, 1], F32)
            nc.sync.dma_start(bias_t[:], cstf[:])
            identb_t = const_pool.tile([128, 128], BF16)
            nc.sync.dma_start(identb_t[:], identb[:])
            ones_t = const_pool.tile([128, 1], BF16)
            nc.sync.dma_start(ones_t[:], ones[:])

            for gi in range(1, len(CHUNKS)):
                t = zt_pool.tile(
                    [128, 2, CHUNKS[gi][1]], F8, tag=f"zt{gi}", name=f"zt{gi}"
                )
                nc.sync.dma_start(t[:], zparams[gi][:])
                zchunk[gi] = t

            tgt_t = stats_pool.tile([128, 4], F32)
            colacc = cacc_pool.tile([128, PADB], F32)

            def chunk_of(gstart):
                for ci, (c0, cw) in enumerate(CHUNKS):
                    if c0 <= gstart < c0 + cw:
                        return ci
                raise AssertionError(gstart)

            def chunk_ap(gstart, width):
                ci = chunk_of(gstart)
                off = gstart - CHUNKS[ci][0]
                assert off + width <= CHUNKS[ci][1], (gstart, width)
                return zchunk[ci][:, :, off:off + width]

            def emit_gram(ps, base, off, w_st, stat_ap):
                loc = 0
                while loc < w_st:
                    seg = min(512, w_st - loc)
                    nc.tensor.matmul(
                        ps[:, loc:loc + seg],
                        lhsT=stat_ap,
                        rhs=chunk_ap(base + off + loc, seg),
                        start=True,
                        stop=True,
                        perf_mode=mybir.MatmulPerfMode.DoubleRow,
                    )
                    loc += seg

            def emit_colsum(echunk, ql, off, w_st):
                for j in range(w_st // 128):
                    kb = ql + (off + j * 128) // 128
                    if kb == ql:
                        continue  # diagonal block: row-sums already cover it
                    nc.tensor.matmul(
                        colacc[:, kb:kb + 1],
                        lhsT=echunk(j),
                        rhs=ones_t[:, 0:1],
                        start=(FIRST_Q[kb] == ql),
                        stop=(LAST_Q[kb] == ql),
                    )

            # PE warm-up + filler: junk matmuls into a dead psum bank keep the
            # PE HAM clock gate at 2.4 GHz (it throttles to 1.2 GHz whenever
            # the activity monitor sees an idle-ish 3.4us window, doubling
            # every gram matmul). Source is a memset tile (no DMA dep).
            wsrc = const_pool.tile([128, 512], BF16)
            nc.gpsimd.memset(wsrc[:], 0)
            warm = warm_pool.tile([128, 512], F32)

            def junk(n):
                for _ in range(n):
                    nc.tensor.matmul(
                        warm[:, :512],
                        lhsT=wsrc[:, 0:128],
                        rhs=wsrc[:],
                        start=True,
                        stop=True,
                    )

            junk(12)

            # Per q-row: 4 psum tenants — 2 ACT strips (exact exp) + 2 DVE
            # sub-strips (exp2 bit-trick). With a 2-slot psum pool the
            # boundary tile's slot is freed by an EARLY DVE tensor_scalar,
            # not the late exp, so ACT runs back-to-back across rows.
            # Late-dependency colsums are carried into the next row's PE
            # stream (colacc first/last flags are per (kb, q-row), so the
            # cross-row emission order per column is preserved).
            carried = []
            for s, (ql, L) in enumerate(zip(QROWS, LS)):
                base = ql * 128
                stat_ap = chunk_ap(base, 128)
                rowq = stats_pool.tile([128, 4], F32, tag=f"rowq{s}")
                w0 = w1 = STRIPW
                rem = L * 128 - 2 * STRIPW          # 1152 or 1024
                # both sub-strip offsets stay 512-aligned w.r.t. the chunk grid
                wD1, wD2 = (1024, 128) if rem == 1152 else (512, 512)
                o0, o1, oD1, oD2 = 0, w0, 2 * STRIPW, 2 * STRIPW + wD1

                # --- ACT strip 0 (boundary: slot freed by prev row's TS1) ---
                ps0 = gram_pool.tile([128, STRIPW], F32, tag="ps", name="ps0")
                emit_gram(ps0, base, o0, w0, stat_ap)
                et0 = e_pool.tile([128, STRIPW], BF16, tag="E", name="et0")
                nc.scalar.activation(
                    et0[:], ps0[:], mybir.ActivationFunctionType.Exp,
                    scale=act_scale, bias=bias_t[:],
                    accum_out=rowq[:, 0:1],
                )

                # --- ACT strip 1 ---
                ps1 = gram_pool.tile([128, STRIPW], F32, tag="ps", name="ps1")
                emit_gram(ps1, base, o1, w1, stat_ap)
                for work in carried:
                    work()
                carried = []
                junk(1)
                et1 = e_pool.tile([128, STRIPW], BF16, tag="E", name="et1")
                nc.scalar.activation(
                    et1[:], ps1[:], mybir.ActivationFunctionType.Exp,
                    scale=act_scale, bias=bias_t[:],
                    accum_out=rowq[:, 1:2],
                )

                # --- DVE sub-strip 2 FIRST (short TS frees its slot fast;
                # that slot is the next row's boundary gram slot) ---
                psD2 = gram_pool.tile([128, STRIPW], F32, tag="ps", name="psD2")
                emit_gram(psD2, base, oD2, wD2, stat_ap)
                e16b = e16_pool.tile([128, 512], I16, tag="E16b")
                nc.vector.tensor_scalar(
                    e16b[:, :wD2], psD2[:, :wD2],
                    TRICK_MUL, TRICK_ADD,
                    mybir.AluOpType.mult, mybir.AluOpType.add,
                )
                nc.vector.tensor_reduce(
                    out=rowq[:, 3:4], in_=e16b[:, :wD2].bitcast(BF16),
                    axis=mybir.AxisListType.X, op=mybir.AluOpType.add,
                )
                emit_colsum(lambda j: et0[:, j * 128:(j + 1) * 128], ql, o0, w0)

                # --- DVE sub-strip 1 (long; spills into the next row) ---
                psD1 = gram_pool.tile([128, STRIPW], F32, tag="ps", name="psD1")
                emit_gram(psD1, base, oD1, wD1, stat_ap)
                e16a = e16_pool.tile([128, 1024], I16, tag="E16a")
                nc.vector.tensor_scalar(
                    e16a[:, :wD1], psD1[:, :wD1],
                    TRICK_MUL, TRICK_ADD,
                    mybir.AluOpType.mult, mybir.AluOpType.add,
                )
                nc.vector.tensor_reduce(
                    out=rowq[:, 2:3], in_=e16a[:, :wD1].bitcast(BF16),
                    axis=mybir.AxisListType.X, op=mybir.AluOpType.add,
                )
                # target pairs: diagonal of tile (q, q+32) = DVE sub-strip 2;
                # s' recovered on host from the exp bits
                if s < 4:
                    dg = d_pool.tile([128, 128], BF16, tag="dg")
                    nc.vector.tensor_mul(
                        dg[:], e16b[:, 0:128].bitcast(BF16), identb_t[:]
                    )
                    nc.vector.tensor_reduce(
                        out=tgt_t[:, s:s + 1], in_=dg[:],
                        axis=mybir.AxisListType.X, op=mybir.AluOpType.add,
                    )

                def make_carry(et1=et1, e16a=e16a, e16b=e16b, ql=ql, o1=o1,
                               w1=w1, oD1=oD1, wD1=wD1, oD2=oD2, wD2=wD2):
                    return [
                        lambda: emit_colsum(
                            lambda j: et1[:, j * 128:(j + 1) * 128], ql, o1, w1
                        ),
                        lambda: emit_colsum(
                            lambda j: e16a[:, j * 128:(j + 1) * 128].bitcast(BF16),
                            ql, oD1, wD1,
                        ),
                        lambda: emit_colsum(
                            lambda j: e16b[:, j * 128:(j + 1) * 128].bitcast(BF16),
                            ql, oD2, wD2,
                        ),
                    ]

                carried = make_carry()

                nc.sync.dma_start(rowout[:, s * 4:(s + 1) * 4], rowq[:])
                if s == 3:
                    nc.sync.dma_start(tgtout[:], tgt_t[:])

            for work in carried:
                work()

            colsb = stats_pool.tile([128, PADB], F32)
            nc.vector.tensor_copy(colsb[:], colacc[:])
            nc.sync.dma_start(colout[:], colsb[:])

    nc.compile()
    return nc


_NC_CACHE: list = []


def _get_nc():
    if not _NC_CACHE:
        _NC_CACHE.append(build_nc())
    return _NC_CACHE[0]


def _prepare_inputs(z1: np.ndarray, z2: np.ndarray) -> list[dict]:
    z = np.concatenate([np.asarray(z1), np.asarray(z2)], axis=0).astype(np.float32)
    zn = z / np.maximum(np.linalg.norm(z, axis=1, keepdims=True), 1e-12)
    # DoubleRow layout: [128 partitions, 2 k-slabs, 8192 keys], scaled by 16
    zt = np.ascontiguousarray((zn.T * ZSCALE).reshape(2, 128, N).transpose(1, 0, 2))
    zt8 = zt.astype(NP_F8)
    ones = np.ones((128, 1), dtype=NP_BF16)
    biasv = np.full((128, 1), -1.0 / TEMP, dtype=np.float32)
    identb = np.eye(128, dtype=NP_BF16)
    in_maps = []
    for c in range(NCORES):
        rolled = np.roll(zt8, -128 * c, axis=2)
        padded = np.concatenate([rolled, rolled[:, :, : PADW - N]], axis=2)
        m = {"cstf": biasv, "identb": identb, "ones": ones}
        for g, (c0, cw) in enumerate(CHUNKS):
            m[f"zt8c{g}"] = np.ascontiguousarray(padded[:, :, c0:c0 + cw])
        in_maps.append(m)
    return in_maps


def _reduce_outputs(results) -> np.float32:
    R = np.zeros(N, dtype=np.float64)
    tgt = np.empty(N, dtype=np.float64)
    for c in range(NCORES):
        rowacc = results[c]["rowout"].astype(np.float64)   # [128, N_SLOTS]
        colacc = results[c]["colout"].astype(np.float64)   # [128, PADB]
        tgt_c = results[c]["tgtout"].astype(np.float64)    # [128, 4]
        for s, ql in enumerate(QROWS):
            g = ql + c   # global q block (<= 63)
            r = rowacc[:, s * 4:(s + 1) * 4].sum(axis=1)
            R[g * 128:(g + 1) * 128] += r
        for kb in range(1, PADB):
            gk = (kb + c) % NB
            R[gk * 128:(gk + 1) * 128] += colacc[:, kb]
        for s in range(4):
            gq = QROWS[s] + c   # < 32
            rows = gq * 128 + np.arange(128)
            # tgt_c holds exp((s'/256-1)/T) bits summed over the diagonal
            sp = 256.0 + 256.0 * TEMP * np.log(np.maximum(tgt_c[:, s], 1e-300))
            tgt[rows] = sp
            tgt[rows + B] = sp
    logS = 1.0 / TEMP + np.log(R)
    loss = np.mean(logS - tgt / (ZSCALE * ZSCALE * TEMP))
    return np.float32(loss)


def _run(z1, z2, trace=False):
    nc = _get_nc()
    in_maps = _prepare_inputs(z1, z2)
    res = run_bass_kernel_spmd(nc, in_maps, list(range(NCORES)), trace=trace)
    return _reduce_outputs(res.results), res


def kernel(z1: np.ndarray, z2: np.ndarray) -> np.ndarray:
    loss, _ = _run(z1, z2, trace=False)
    return loss


if __name__ == "__main__":
    rng = np.random.default_rng(0)
    z1 = rng.standard_normal((B, D), dtype=np.float32)
    z2 = rng.standard_normal((B, D), dtype=np.float32)
    print(kernel(z1, z2))


# revision 12
# speedup vs baseline: 1.0431x; 1.0007x over previous
"""NT-Xent contrastive loss (SimCLR) on 8 Trainium2 NeuronCores.

z1, z2 [4096, 256] fp32 -> scalar loss.
  zn = l2norm(concat(z1, z2))            [8192, 256]
  sim = zn @ zn.T / 0.07                 [8192, 8192]
  loss = -mean_i log_softmax(sim)[i, (i + 4096) % 8192]

Exploits the symmetry of exp(sim): each unordered 128x128 block pair is
computed ONCE (half the matmul + half the exp work). Row sums come from the
free axis (ACT exp accumulator / DVE reduce); the transpose-direction sums
come from PE matmuls with the exp tile as stationary operand against a ones
vector (partition-axis reduction into a persistent PSUM accumulator).

Work distribution (circulant, SPMD-uniform): 64 key blocks; global block row
q covers ring range [q, q+L) mod 64 with L=33 for q<32 else 32 — every
unordered pair covered exactly once (2080 tiles). Core c owns global rows
{c, c+8, ..., c+56}; after rolling the key space by 128*c columns every core
runs the IDENTICAL program on local q-rows {0,8,...,56}. Keys are
ring-unrolled (padded to 88 blocks) and supplied as contiguous chunk
tensors; chunk 0 is small so compute starts early.

Per q-row, four PSUM tenants in a 2-slot pool: 2 ACT strips (exact exp via
the scalar engine, fused row-sum accumulate) + 2 DVE sub-strips computed
with an int16/bf16 exp2 bit-trick (tail terms only; ~3% per-term error on
<1% of the softmax mass). The short DVE sub-strip's tensor_scalar frees the
boundary slot early so the ACT engine runs nearly back-to-back across rows;
late-dependency colsum matmuls are carried into the next row's PE stream.
Junk matmuls into a dead PSUM bank keep the PE HAM clock gate at 2.4 GHz.

Gram matmuls: fp8 e4m3 (z pre-scaled by 16), DoubleRow perf mode (K=256 in
one pass). Overall rel err ~1.1e-3 (tolerance 2e-2), dominated by the fp8
quantization of z.

Per-core outputs: rowout [128,32] strip row-sums, colout [128,88] col-sum
partials, tgtout [128,4] target-pair exp values. Host sums partials across
cores, takes logs, and averages (tiny: ~100 KB total).
"""

import math

import numpy as np

import concourse.bass as bass
import concourse.tile as tile
from concourse import bacc, mybir
from concourse.bass_utils import run_bass_kernel_spmd

B, D = 4096, 256
N = 2 * B
NCORES = 8
NB = N // 128                 # 64 key blocks
TEMP = 0.07
ZSCALE = 16.0                 # z pre-scaled by 16 -> s' = 256 * s_true

QROWS = [0, 8, 16, 24, 32, 40, 48, 56]   # local q-row blocks (all cores)
LS = [33, 33, 33, 33, 32, 32, 32, 32]    # ring length (blocks) per slot
PADB = 88                                 # local key blocks incl ring pad
PADW = PADB * 128                         # 11264 padded key cols
CHUNKS = [(0, 1536), (1536, 1536), (3072, 3072), (6144, 3072), (9216, 2048)]

F32 = mybir.dt.float32
BF16 = mybir.dt.bfloat16
I16 = mybir.dt.int16
F8 = mybir.dt.float8e4
NP_F8 = mybir.dt.np(F8)
NP_BF16 = mybir.dt.np(BF16)

STRIPW = 1536

# exp2 bit-trick: round(MUL*s' + ADD) as int16, bitcast bf16 ~=
# exp((s'/256-1)/T); rel err within +-4%, mean ~1e-4 — tail strips only.
_LOG2E = math.log2(math.e)
TRICK_MUL = 128.0 * _LOG2E / (256.0 * TEMP)
TRICK_ADD = 128.0 * (126.94269504 - _LOG2E / TEMP)



N_SLOTS = 4 * len(QROWS)  # 4 row-sum slots per q-row


def contrib_tables():
    first, last = {}, {}
    for ql, L in zip(QROWS, LS):
        for kb in range(ql + 1, ql + L):
            if kb not in first:
                first[kb] = ql
            last[kb] = ql
    assert sorted(first) == list(range(1, PADB))
    return first, last


FIRST_Q, LAST_Q = contrib_tables()


def build_nc() -> bass.Bass:
    nc = bacc.Bacc("TRN2", target_bir_lowering=False, debug=False, num_devices=NCORES)
    zparams = [
        nc.declare_dram_parameter(f"zt8c{g}", [128, 2, cw], F8, isOutput=False)
        for g, (c0, cw) in enumerate(CHUNKS)
    ]
    cstf = nc.declare_dram_parameter("cstf", [128, 1], F32, isOutput=False)
    identb = nc.declare_dram_parameter("identb", [128, 128], BF16, isOutput=False)
    ones = nc.declare_dram_parameter("ones", [128, 1], BF16, isOutput=False)
    rowout = nc.declare_dram_parameter("rowout", [128, N_SLOTS], F32, isOutput=True)
    tgtout = nc.declare_dram_parameter("tgtout", [128, 4], F32, isOutput=True)
    colout = nc.declare_dram_parameter("colout", [128, PADB], F32, isOutput=True)

    act_scale = 1.0 / (ZSCALE * ZSCALE * TEMP)

    with tile.TileContext(nc) as tc:
        with (
            tc.tile_pool(name="zt", bufs=1) as zt_pool,
            tc.tile_pool(name="const", bufs=1) as const_pool,
            tc.tile_pool(name="stats", bufs=1) as stats_pool,
            tc.tile_pool(name="escr", bufs=3) as e_pool,
            tc.tile_pool(name="e16", bufs=3) as e16_pool,
            tc.tile_pool(name="dscr", bufs=2) as d_pool,
            tc.tile_pool(name="gram", bufs=2, space="PSUM") as gram_pool,
            tc.tile_pool(name="cacc", bufs=1, space="PSUM") as cacc_pool,
            tc.tile_pool(name="warmp", bufs=1, space="PSUM") as warm_pool,
        ):
            # chunk 0 first: compute can't start without it
            zchunk = [None] * len(CHUNKS)
            zchunk[0] = zt_pool.tile([128, 2, CHUNKS[0][1]], F8, tag="zt0", name="zt0")
            nc.sync.dma_start(zchunk[0][:], zparams[0][:])

            bias_t = const_pool.tile([128, 1], F32)
            nc.sync.dma_start(bias_t[:], cstf[:])
            identb_t = const_pool.tile([128, 128], BF16)
            nc.sync.dma_start(identb_t[:], identb[:])
            ones_t = const_pool.tile([128, 1], BF16)
            nc.sync.dma_start(ones_t[:], ones[:])

            for gi in range(1, len(CHUNKS)):
                t = zt_pool.tile(
                    [128, 2, CHUNKS[gi][1]], F8, tag=f"zt{gi}", name=f"zt{gi}"
                )
                nc.sync.dma_start(t[:], zparams[gi][:])
                zchunk[gi] = t

            tgt_t = stats_pool.tile([128, 4], F32)
            colacc = cacc_pool.tile([128, PADB], F32)

            def chunk_of(gstart):
                for ci, (c0, cw) in enumerate(CHUNKS):
                    if c0 <= gstart < c0 + cw:
                        return ci
                raise AssertionError(gstart)

            def chunk_ap(gstart, width):
                ci = chunk_of(gstart)
                off = gstart - CHUNKS[ci][0]
                assert off + width <= CHUNKS[ci][1], (gstart, width)
                return zchunk[ci][:, :, off:off + width]

            def emit_gram(ps, base, off, w_st, stat_ap):
                loc = 0
                while loc < w_st:
                    seg = min(512, w_st - loc)
                    nc.tensor.matmul(
                        ps[:, loc:loc + seg],
                        lhsT=stat_ap,
                        rhs=chunk_ap(base + off + loc, seg),
                        start=True,
                        stop=True,
                        perf_mode=mybir.MatmulPerfMode.DoubleRow,
                    )
                    loc += seg

            def emit_colsum(echunk, ql, off, w_st):
                for j in range(w_st // 128):
                    kb = ql + (off + j * 128) // 128
                    if kb == ql:
                        continue  # diagonal block: row-sums already cover it
                    nc.tensor.matmul(
                        colacc[:, kb:kb + 1],
                        lhsT=echunk(j),
                        rhs=ones_t[:, 0:1],
                        start=(FIRST_Q[kb] == ql),
                        stop=(LAST_Q[kb] == ql),
                    )

            # PE warm-up + filler: junk matmuls into a dead psum bank keep the
            # PE HAM clock gate at 2.4 GHz (it throttles to 1.2 GHz whenever
            # the activity monitor sees an idle-ish 3.4us window, doubling
            # every gram matmul). Source is a memset tile (no DMA dep).
            wsrc = const_pool.tile([128, 512], BF16)
            nc.gpsimd.memset(wsrc[:], 0)
            warm = warm_pool.tile([128, 512], F32)

            def junk(n):
                for _ in range(n):
                    nc.tensor.matmul(
                        warm[:, :512],
                        lhsT=wsrc[:, 0:128],
                        rhs=wsrc[:],
                        start=True,
                        stop=True,
                    )

            junk(12)

            # Per q-row: 4 psum tenants — 2 ACT strips (exact exp) + 2 DVE
            # sub-strips (exp2 bit-trick). With a 2-slot psum pool the
            # boundary tile's slot is freed by an EARLY DVE tensor_scalar,
            # not the late exp, so ACT runs back-to-back across rows.
            # Late-dependency colsums are carried into the next row's PE
            # stream (colacc first/last flags are per (kb, q-row), so the
            # cross-row emission order per column is preserved).
            carried = []
            for s, (ql, L) in enumerate(zip(QROWS, LS)):
                base = ql * 128
                stat_ap = chunk_ap(base, 128)
                rowq = stats_pool.tile([128, 4], F32, tag=f"rowq{s}")
                w0 = w1 = STRIPW
                rem = L * 128 - 2 * STRIPW          # 1152 or 1024
                # both sub-strip offsets stay 512-aligned w.r.t. the chunk grid
                wD1, wD2 = (1024, 128) if rem == 1152 else (512, 512)
                o0, o1, oD1, oD2 = 0, w0, 2 * STRIPW, 2 * STRIPW + wD1

                # --- ACT strip 0 (boundary: slot freed by prev row's TS1) ---
                ps0 = gram_pool.tile([128, STRIPW], F32, tag="ps", name="ps0")
                emit_gram(ps0, base, o0, w0, stat_ap)
                et0 = e_pool.tile([128, STRIPW], BF16, tag="E", name="et0")
                nc.scalar.activation(
                    et0[:], ps0[:], mybir.ActivationFunctionType.Exp,
                    scale=act_scale, bias=bias_t[:],
                    accum_out=rowq[:, 0:1],
                )

                # --- ACT strip 1 ---
                ps1 = gram_pool.tile([128, STRIPW], F32, tag="ps", name="ps1")
                emit_gram(ps1, base, o1, w1, stat_ap)
                for work in carried:
                    work()
                carried = []
                junk(1)
                et1 = e_pool.tile([128, STRIPW], BF16, tag="E", name="et1")
                nc.scalar.activation(
                    et1[:], ps1[:], mybir.ActivationFunctionType.Exp,
                    scale=act_scale, bias=bias_t[:],
                    accum_out=rowq[:, 1:2],
                )

                # --- DVE sub-strip 2 FIRST (short TS frees its slot fast;
                # that slot is the next row's boundary gram slot) ---
                psD2 = gram_pool.tile([128, STRIPW], F32, tag="ps", name="psD2")
                emit_gram(psD2, base, oD2, wD2, stat_ap)
                e16b = e16_pool.tile([128, 512], I16, tag="E16b")
                nc.vector.tensor_scalar(
                    e16b[:, :wD2], psD2[:, :wD2],
                    TRICK_MUL, TRICK_ADD,
                    mybir.AluOpType.mult, mybir.AluOpType.add,
                )
                nc.vector.tensor_reduce(
                    out=rowq[:, 3:4], in_=e16b[:, :wD2].bitcast(BF16),
                    axis=mybir.AxisListType.X, op=mybir.AluOpType.add,
                )
                emit_colsum(lambda j: et0[:, j * 128:(j + 1) * 128], ql, o0, w0)

                # --- DVE sub-strip 1 (long; spills into the next row) ---
                psD1 = gram_pool.tile([128, STRIPW], F32, tag="ps", name="psD1")
                emit_gram(psD1, base, oD1, wD1, stat_ap)
                e16a = e16_pool.tile([128, 1024], I16, tag="E16a")
                nc.vector.tensor_scalar(
                    e16a[:, :wD1], psD1[:, :wD1],
                    TRICK_MUL, TRICK_ADD,
                    mybir.AluOpType.mult, mybir.AluOpType.add,
                )
                nc.vector.tensor_reduce(
                    out=rowq[:, 2:3], in_=e16a[:, :wD1].bitcast(BF16),
                    axis=mybir.AxisListType.X, op=mybir.AluOpType.add,
                )
                # target pairs: diagonal of tile (q, q+32) = DVE sub-strip 2;
                # s' recovered on host from the exp bits
                if s < 4:
                    dg = d_pool.tile([128, 128], BF16, tag="dg")
                    nc.vector.tensor_mul(
                        dg[:], e16b[:, 0:128].bitcast(BF16), identb_t[:]
                    )
                    nc.vector.tensor_reduce(
                        out=tgt_t[:, s:s + 1], in_=dg[:],
                        axis=mybir.AxisListType.X, op=mybir.AluOpType.add,
                    )

                def make_carry(et1=et1, e16a=e16a, e16b=e16b, ql=ql, o1=o1,
                               w1=w1, oD1=oD1, wD1=wD1, oD2=oD2, wD2=wD2):
                    return [
                        lambda: emit_colsum(
                            lambda j: et1[:, j * 128:(j + 1) * 128], ql, o1, w1
                        ),
                        lambda: emit_colsum(
                            lambda j: e16a[:, j * 128:(j + 1) * 128].bitcast(BF16),
                            ql, oD1, wD1,
                        ),
                        lambda: emit_colsum(
                            lambda j: e16b[:, j * 128:(j + 1) * 128].bitcast(BF16),
                            ql, oD2, wD2,
                        ),
                    ]

                carried = make_carry()

                nc.sync.dma_start(rowout[:, s * 4:(s + 1) * 4], rowq[:])
                if s == 3:
                    nc.sync.dma_start(tgtout[:], tgt_t[:])

            for work in carried:
                work()

            colsb = stats_pool.tile([128, PADB], F32)
            nc.vector.tensor_copy(colsb[:], colacc[:])
            nc.sync.dma_start(colout[:], colsb[:])

    nc.compile()
    return nc


_NC_CACHE: list = []


def _get_nc():
    if not _NC_CACHE:
        _NC_CACHE.append(build_nc())
    return _NC_CACHE[0]


def _prepare_inputs(z1: np.ndarray, z2: np.ndarray) -> list[dict]:
    z = np.concatenate([np.asarray(z1), np.asarray(z2)], axis=0).astype(np.float32)
    zn = z / np.maximum(np.linalg.norm(z, axis=1, keepdims=True), 1e-12)
    # DoubleRow layout: [128 partitions, 2 k-slabs, 8192 keys], scaled by 16
    zt = np.ascontiguousarray((zn.T * ZSCALE).reshape(2, 128, N).transpose(1, 0, 2))
    zt8 = zt.astype(NP_F8)
    ones = np.ones((128, 1), dtype=NP_BF16)
    biasv = np.full((128, 1), -1.0 / TEMP, dtype=np.float32)
    identb = np.eye(128, dtype=NP_BF16)
    in_maps = []
    for c in range(NCORES):
        rolled = np.roll(zt8, -128 * c, axis=2)
        padded = np.concatenate([rolled, rolled[:, :, : PADW - N]], axis=2)
        m = {"cstf": biasv, "identb": identb, "ones": ones}
        for g, (c0, cw) in enumerate(CHUNKS):
            m[f"zt8c{g}"] = np.ascontiguousarray(padded[:, :, c0:c0 + cw])
        in_maps.append(m)
    return in_maps


def _reduce_outputs(results) -> np.float32:
    R = np.zeros(N, dtype=np.float64)
    tgt = np.empty(N, dtype=np.float64)
    for c in range(NCORES):
        rowacc = results[c]["rowout"].astype(np.float64)   # [128, N_SLOTS]
        colacc = results[c]["colout"].astype(np.float64)   # [128, PADB]
        tgt_c = results[c]["tgtout"].astype(np.float64)    # [128, 4]
        for s, ql in enumerate(QROWS):
            g = ql + c   # global q block (<= 63)
            r = rowacc[:, s * 4:(s + 1) * 4].sum(axis=1)
            R[g * 128:(g + 1) * 128] += r
        for kb in range(1, PADB):
            gk = (kb + c) % NB
            R[gk * 128:(gk + 1) * 128] += colacc[:, kb]
        for s in range(4):
            gq = QROWS[s] + c   # < 32
            rows = gq * 128 + np.arange(128)
            # tgt_c holds exp((s'/256-1)/T) bits summed over the diagonal
            sp = 256.0 + 256.0 * TEMP * np.log(np.maximum(tgt_c[:, s], 1e-300))
            tgt[rows] = sp
            tgt[rows + B] = sp
    logS = 1.0 / TEMP + np.log(R)
    loss = np.mean(logS - tgt / (ZSCALE * ZSCALE * TEMP))
    return np.float32(loss)


def _run(z1, z2, trace=False):
    nc = _get_nc()
    in_maps = _prepare_inputs(z1, z2)
    res = run_bass_kernel_spmd(nc, in_maps, list(range(NCORES)), trace=trace)
    return _reduce_outputs(res.results), res


def kernel(z1: np.ndarray, z2: np.ndarray) -> np.ndarray:
    loss, _ = _run(z1, z2, trace=False)
    return loss


if __name__ == "__main__":
    rng = np.random.default_rng(0)
    z1 = rng.standard_normal((B, D), dtype=np.float32)
    z2 = rng.standard_normal((B, D), dtype=np.float32)
    print(kernel(z1, z2))
